# revision 8
# baseline (speedup 1.0000x reference)
"""Trainium2 Bass kernel for nn_ColRepeatCausalLinear.

Math: reference computes out = x @ W + bias with
    W[s, t] = v[t] * d^(t-s)  for t >= s, else 0,   d = clip(decay_value, 0.9, 1)
which factorizes as a decayed prefix scan along S:
    y[b, e, t] = d * y[b, e, t-1] + x[b, e, t]
    out[b, e, t] = v[t] * y[b, e, t] + bias[t]
i.e. O(B*E*S) work instead of the O(B*E*S^2) dense matmul.

Mapping: data-parallel over B across 8 NeuronCores (x[b] per core, params
replicated). Per core, tiles of (128 rows, S=2048) stay in natural layout
(E on partitions, S on the free axis); the recurrence runs on the Vector
engine's TensorTensorScan instruction, followed by one tensor_tensor mult
by a broadcast v row. DMA-bound by design (~16 MB HBM traffic per core).

Hardcoded problem shapes: x (8, 1024, 2048) f32, weight (1, 2048),
bias (2048,), decay_value (1,).
"""

import numpy as np

import concourse.bacc as bacc
import concourse.mybir as mybir
from concourse.tile import TileContext
from concourse.bass_utils import run_bass_kernel_spmd

B, E, S = 8, 1024, 2048
P = 128
N_CORES = 8
F32 = mybir.dt.float32

_cache = {}


def _build(d: float, has_bias: bool):
    nc = bacc.Bacc(
        "TRN2",
        target_bir_lowering=False,
        debug=False,
        enable_asserts=False,
    )
    x = nc.dram_tensor("x", [E, S], F32, kind="ExternalInput").ap()
    vb_dram = nc.dram_tensor("vb", [P, S], F32, kind="ExternalInput").ap()
    bias_dram = None
    if has_bias:
        bias_dram = nc.dram_tensor("biasb", [P, S], F32, kind="ExternalInput").ap()
    out = nc.dram_tensor("out", [E, S], F32, kind="ExternalOutput").ap()

    with TileContext(nc) as tc:
        with (
            tc.tile_pool(name="const", bufs=1) as cpool,
            tc.tile_pool(name="xs", bufs=6) as xpool,
            tc.tile_pool(name="ys", bufs=2) as ypool,
            tc.tile_pool(name="os", bufs=4) as opool,
        ):
            # decay operand: [P, 1] column broadcast along the free axis
            dtile = cpool.tile([P, 1], F32)
            nc.gpsimd.memset(dtile[:], d)
            dbcast = dtile[:].broadcast_to([P, S])
            # Loads go on the SP HWDGE ring (nc.sync), stores on the ACT
            # HWDGE ring (nc.scalar): each ring is FIFO, so splitting
            # doubles DMA issue parallelism. vb rides the store ring since
            # nothing needs it until the first mult.
            vb = cpool.tile([P, S], F32)
            nc.scalar.dma_start(out=vb[:], in_=vb_dram)
            if has_bias:
                bb = cpool.tile([P, S], F32)
                nc.scalar.dma_start(out=bb[:], in_=bias_dram)
            H = S // 2
            n_tiles = E // P
            for i in range(n_tiles):
                xt = xpool.tile([P, S], F32)
                yt = ypool.tile([P, S], F32)
                if i == 0:
                    # First tile: halves on BOTH rings in parallel, so the
                    # first scan starts ~3us earlier; chain the half-scans.
                    nc.sync.dma_start(out=xt[:, :H], in_=x[:P, :H])
                    nc.scalar.dma_start(out=xt[:, H:], in_=x[:P, H:])
                    nc.vector.tensor_tensor_scan(
                        yt[:, :H], dtile[:].broadcast_to([P, H]), xt[:, :H],
                        0.0, mybir.AluOpType.mult, mybir.AluOpType.add,
                    )
                    nc.vector.tensor_tensor_scan(
                        yt[:, H:], dtile[:].broadcast_to([P, H]), xt[:, H:],
                        yt[:, H - 1 : H], mybir.AluOpType.mult, mybir.AluOpType.add,
                    )
                else:
                    nc.sync.dma_start(out=xt[:], in_=x[i * P : (i + 1) * P, :])
                    nc.vector.tensor_tensor_scan(
                        yt[:], dbcast, xt[:],
                        0.0, mybir.AluOpType.mult, mybir.AluOpType.add,
                    )
                ot = opool.tile([P, S], F32)
                if i == n_tiles - 1:
                    # Last tile: split mult; store halves on both rings so the
                    # tail overlaps.
                    nc.vector.tensor_mul(ot[:, :H], yt[:, :H], vb[:, :H])
                    if has_bias:
                        nc.vector.tensor_add(ot[:, :H], ot[:, :H], bb[:, :H])
                    nc.scalar.dma_start(
                        out=out[i * P : (i + 1) * P, :H], in_=ot[:, :H]
                    )
                    nc.vector.tensor_mul(ot[:, H:], yt[:, H:], vb[:, H:])
                    if has_bias:
                        nc.vector.tensor_add(ot[:, H:], ot[:, H:], bb[:, H:])
                    nc.sync.dma_start(
                        out=out[i * P : (i + 1) * P, H:], in_=ot[:, H:]
                    )
                else:
                    nc.vector.tensor_mul(ot[:], yt[:], vb[:])
                    if has_bias:
                        nc.vector.tensor_add(ot[:], ot[:], bb[:])
                    nc.scalar.dma_start(out=out[i * P : (i + 1) * P, :], in_=ot[:])
    nc.compile()
    return nc


def _run(x, weight, bias, decay_value, trace=False):
    x = np.asarray(x, dtype=np.float32)
    weight = np.asarray(weight, dtype=np.float32)
    bias = np.asarray(bias, dtype=np.float32)
    decay_value = np.asarray(decay_value)
    assert x.shape == (B, E, S), x.shape

    # DECAY_CONSTANT = 1.0 in the reference; exponent is (t - s) / 1.0.
    d = float(np.clip(np.float64(decay_value.reshape(-1)[0]), 0.9, 1.0))
    has_bias = bool(np.any(bias))

    key = (d, has_bias)
    if key not in _cache:
        _cache[key] = _build(d, has_bias)
    nc = _cache[key]

    vb = np.ascontiguousarray(
        np.broadcast_to(weight.reshape(1, S), (P, S)), dtype=np.float32
    )
    bb = None
    if has_bias:
        bb = np.ascontiguousarray(
            np.broadcast_to(bias.reshape(1, S), (P, S)), dtype=np.float32
        )

    in_maps = []
    for b in range(N_CORES):
        m = {"x": np.ascontiguousarray(x[b]), "vb": vb}
        if has_bias:
            m["biasb"] = bb
        in_maps.append(m)

    res = run_bass_kernel_spmd(
        nc, in_maps, core_ids=list(range(N_CORES)), trace=trace
    )
    out = np.stack([r["out"] for r in res.results], axis=0)
    return out, res


def kernel(x, weight, bias, decay_value):
    out, _ = _run(x, weight, bias, decay_value)
    return out


# revision 9
# speedup vs baseline: 1.0843x; 1.0843x over previous
"""Trainium2 Bass kernel for nn_ColRepeatCausalLinear.

Math: reference computes out = x @ W + bias with
    W[s, t] = v[t] * d^(t-s)  for t >= s, else 0,   d = clip(decay_value, 0.9, 1)
which factorizes as a decayed prefix scan along S:
    y[b, e, t] = d * y[b, e, t-1] + x[b, e, t]
    out[b, e, t] = v[t] * y[b, e, t] + bias[t]
i.e. O(B*E*S) work instead of the O(B*E*S^2) dense matmul.

Mapping: data-parallel over B across 8 NeuronCores (x[b] per core, params
replicated). Per core, tiles of (128 rows, S=2048) stay in natural layout
(E on partitions, S on the free axis); the recurrence runs on the Vector
engine's TensorTensorScan instruction, followed by one tensor_tensor mult
by a broadcast v row. DMA-bound by design (~16 MB HBM traffic per core).

Hardcoded problem shapes: x (8, 1024, 2048) f32, weight (1, 2048),
bias (2048,), decay_value (1,).
"""

import numpy as np

import concourse.bacc as bacc
import concourse.mybir as mybir
from concourse.tile import TileContext
from concourse.bass_utils import run_bass_kernel_spmd

B, E, S = 8, 1024, 2048
P = 128
N_CORES = 8
F32 = mybir.dt.float32

_cache = {}


def _build(d: float, has_bias: bool):
    nc = bacc.Bacc(
        "TRN2",
        target_bir_lowering=False,
        debug=False,
        enable_asserts=False,
    )
    x = nc.dram_tensor("x", [E, S], F32, kind="ExternalInput").ap()
    vb_dram = nc.dram_tensor("vb", [P, S], F32, kind="ExternalInput").ap()
    bias_dram = None
    if has_bias:
        bias_dram = nc.dram_tensor("biasb", [P, S], F32, kind="ExternalInput").ap()
    out = nc.dram_tensor("out", [E, S], F32, kind="ExternalOutput").ap()

    with TileContext(nc) as tc:
        with (
            tc.tile_pool(name="const", bufs=1) as cpool,
            tc.tile_pool(name="xs", bufs=6) as xpool,
            tc.tile_pool(name="ys", bufs=2) as ypool,
            tc.tile_pool(name="os", bufs=4) as opool,
        ):
            # decay operand: [P, 1] column broadcast along the free axis
            dtile = cpool.tile([P, 1], F32)
            nc.gpsimd.memset(dtile[:], d)
            dbcast = dtile[:].broadcast_to([P, S])
            # Loads go on the SP HWDGE ring (nc.sync), stores on the ACT
            # HWDGE ring (nc.scalar): each ring is FIFO, so splitting
            # doubles DMA issue parallelism.
            H = S // 2
            n_tiles = E // P
            vb = cpool.tile([P, S], F32)
            if has_bias:
                bb = cpool.tile([P, S], F32)
            for i in range(n_tiles):
                xt = xpool.tile([P, S], F32)
                if i == 0:
                    # First tile: halves on BOTH rings in parallel so the
                    # first scan starts ~3.5us earlier.
                    nc.sync.dma_start(out=xt[:, :H], in_=x[:P, :H])
                    nc.scalar.dma_start(out=xt[:, H:], in_=x[:P, H:])
                    # vb queues behind the first half-tile; needed only by
                    # the first mult.
                    nc.scalar.dma_start(out=vb[:], in_=vb_dram)
                    if has_bias:
                        nc.scalar.dma_start(out=bb[:], in_=bias_dram)
                else:
                    nc.sync.dma_start(out=xt[:], in_=x[i * P : (i + 1) * P, :])
                yt = ypool.tile([P, S], F32)
                nc.vector.tensor_tensor_scan(
                    yt[:], dbcast, xt[:],
                    0.0, mybir.AluOpType.mult, mybir.AluOpType.add,
                )
                ot = opool.tile([P, S], F32)
                nc.vector.tensor_mul(ot[:], yt[:], vb[:])
                if has_bias:
                    nc.vector.tensor_add(ot[:], ot[:], bb[:])
                nc.scalar.dma_start(out=out[i * P : (i + 1) * P, :], in_=ot[:])
    nc.compile()
    return nc


def _run(x, weight, bias, decay_value, trace=False):
    x = np.asarray(x, dtype=np.float32)
    weight = np.asarray(weight, dtype=np.float32)
    bias = np.asarray(bias, dtype=np.float32)
    decay_value = np.asarray(decay_value)
    assert x.shape == (B, E, S), x.shape

    # DECAY_CONSTANT = 1.0 in the reference; exponent is (t - s) / 1.0.
    d = float(np.clip(np.float64(decay_value.reshape(-1)[0]), 0.9, 1.0))
    has_bias = bool(np.any(bias))

    key = (d, has_bias)
    if key not in _cache:
        _cache[key] = _build(d, has_bias)
    nc = _cache[key]

    vb = np.ascontiguousarray(
        np.broadcast_to(weight.reshape(1, S), (P, S)), dtype=np.float32
    )
    bb = None
    if has_bias:
        bb = np.ascontiguousarray(
            np.broadcast_to(bias.reshape(1, S), (P, S)), dtype=np.float32
        )

    in_maps = []
    for b in range(N_CORES):
        m = {"x": np.ascontiguousarray(x[b]), "vb": vb}
        if has_bias:
            m["biasb"] = bb
        in_maps.append(m)

    res = run_bass_kernel_spmd(
        nc, in_maps, core_ids=list(range(N_CORES)), trace=trace
    )
    out = np.stack([r["out"] for r in res.results], axis=0)
    return out, res


def kernel(x, weight, bias, decay_value):
    out, _ = _run(x, weight, bias, decay_value)
    return out


# revision 10
# speedup vs baseline: 1.1309x; 1.0430x over previous
"""Trainium2 Bass kernel for nn_ColRepeatCausalLinear.

Math: reference computes out = x @ W + bias with
    W[s, t] = v[t] * d^(t-s)  for t >= s, else 0,   d = clip(decay_value, 0.9, 1)
which factorizes as a decayed prefix scan along S:
    y[b, e, t] = d * y[b, e, t-1] + x[b, e, t]
    out[b, e, t] = v[t] * y[b, e, t] + bias[t]
i.e. O(B*E*S) work instead of the O(B*E*S^2) dense matmul.

Mapping: data-parallel over B across 8 NeuronCores (x[b] per core, params
replicated). Per core, tiles of (128 rows, S=2048) stay in natural layout
(E on partitions, S on the free axis); the recurrence runs on the Vector
engine's TensorTensorScan instruction, followed by one tensor_tensor mult
by a broadcast v row. DMA-bound by design (~16 MB HBM traffic per core).

Hardcoded problem shapes: x (8, 1024, 2048) f32, weight (1, 2048),
bias (2048,), decay_value (1,).
"""

import numpy as np

import concourse.bacc as bacc
import concourse.mybir as mybir
from concourse.tile import TileContext
from concourse.bass_utils import run_bass_kernel_spmd

B, E, S = 8, 1024, 2048
P = 128
N_CORES = 8
F32 = mybir.dt.float32

_cache = {}


def _build(d: float, has_bias: bool):
    nc = bacc.Bacc(
        "TRN2",
        target_bir_lowering=False,
        debug=False,
        enable_asserts=False,
    )
    x = nc.dram_tensor("x", [E, S], F32, kind="ExternalInput").ap()
    vb_dram = nc.dram_tensor("vb", [P, S], F32, kind="ExternalInput").ap()
    bias_dram = None
    if has_bias:
        bias_dram = nc.dram_tensor("biasb", [P, S], F32, kind="ExternalInput").ap()
    out = nc.dram_tensor("out", [E, S], F32, kind="ExternalOutput").ap()

    with TileContext(nc) as tc:
        with (
            tc.tile_pool(name="const", bufs=1) as cpool,
            tc.tile_pool(name="xs", bufs=6) as xpool,
            tc.tile_pool(name="ys", bufs=2) as ypool,
            tc.tile_pool(name="os", bufs=4) as opool,
        ):
            # decay operand: [P, 1] column broadcast along the free axis
            dtile = cpool.tile([P, 1], F32)
            nc.gpsimd.memset(dtile[:], d)
            dbcast = dtile[:].broadcast_to([P, S])
            # Loads go on the SP HWDGE ring (nc.sync), stores on the ACT
            # HWDGE ring (nc.scalar): each ring is FIFO, so splitting
            # doubles DMA issue parallelism.
            H = S // 2
            n_tiles = E // P
            vb = cpool.tile([P, S], F32)
            if has_bias:
                bb = cpool.tile([P, S], F32)
            for i in range(n_tiles):
                xt = xpool.tile([P, S], F32)
                nc.sync.dma_start(out=xt[:], in_=x[i * P : (i + 1) * P, :])
                if i == 0:
                    nc.scalar.dma_start(out=vb[:], in_=vb_dram)
                    if has_bias:
                        nc.scalar.dma_start(out=bb[:], in_=bias_dram)
                yt = ypool.tile([P, S], F32)
                nc.vector.tensor_tensor_scan(
                    yt[:], dbcast, xt[:],
                    0.0, mybir.AluOpType.mult, mybir.AluOpType.add,
                )
                ot = opool.tile([P, S], F32)
                if i == n_tiles - 1:
                    # Last tile: split the mult so each half-store (on its
                    # own HWDGE ring) starts as soon as its half is ready.
                    nc.vector.tensor_mul(ot[:, :H], yt[:, :H], vb[:, :H])
                    if has_bias:
                        nc.vector.tensor_add(ot[:, :H], ot[:, :H], bb[:, :H])
                    nc.scalar.dma_start(
                        out=out[i * P : (i + 1) * P, :H], in_=ot[:, :H]
                    )
                    nc.vector.tensor_mul(ot[:, H:], yt[:, H:], vb[:, H:])
                    if has_bias:
                        nc.vector.tensor_add(ot[:, H:], ot[:, H:], bb[:, H:])
                    nc.sync.dma_start(
                        out=out[i * P : (i + 1) * P, H:], in_=ot[:, H:]
                    )
                else:
                    nc.vector.tensor_mul(ot[:], yt[:], vb[:])
                    if has_bias:
                        nc.vector.tensor_add(ot[:], ot[:], bb[:])
                    nc.scalar.dma_start(out=out[i * P : (i + 1) * P, :], in_=ot[:])
    nc.compile()
    return nc


def _run(x, weight, bias, decay_value, trace=False):
    x = np.asarray(x, dtype=np.float32)
    weight = np.asarray(weight, dtype=np.float32)
    bias = np.asarray(bias, dtype=np.float32)
    decay_value = np.asarray(decay_value)
    assert x.shape == (B, E, S), x.shape

    # DECAY_CONSTANT = 1.0 in the reference; exponent is (t - s) / 1.0.
    d = float(np.clip(np.float64(decay_value.reshape(-1)[0]), 0.9, 1.0))
    has_bias = bool(np.any(bias))

    key = (d, has_bias)
    if key not in _cache:
        _cache[key] = _build(d, has_bias)
    nc = _cache[key]

    vb = np.ascontiguousarray(
        np.broadcast_to(weight.reshape(1, S), (P, S)), dtype=np.float32
    )
    bb = None
    if has_bias:
        bb = np.ascontiguousarray(
            np.broadcast_to(bias.reshape(1, S), (P, S)), dtype=np.float32
        )

    in_maps = []
    for b in range(N_CORES):
        m = {"x": np.ascontiguousarray(x[b]), "vb": vb}
        if has_bias:
            m["biasb"] = bb
        in_maps.append(m)

    res = run_bass_kernel_spmd(
        nc, in_maps, core_ids=list(range(N_CORES)), trace=trace
    )
    out = np.stack([r["out"] for r in res.results], axis=0)
    return out, res


def kernel(x, weight, bias, decay_value):
    out, _ = _run(x, weight, bias, decay_value)
    return out
